# revision 1
# baseline (speedup 1.0000x reference)
"""GCN autoencoder (encoder -> GCNConv -> decoder) on 8 TRN2 NeuronCores, v4.

Key idea vs the old SBUF-transpose-gather baseline: message passing gathers
512-byte PAIR rows (two adjacent nodes' bf16 features) straight from the
DRAM table with non-transpose dma_gather.  512B descriptors run ~4.7x
faster per descriptor than 256B ones on TRN2's SWDGE path, and pair ids
(node_position // 2 < 25088) fit int16 without splitting the table.

Per core c (nodes sharded 6250 real + 22 zero-pad = 6272 = 49*128 per core):
  1. encoder:  enc[n] = dinv[n] * relu(x[n] @ W_enc + b_enc)  (bf16 matmuls,
     f32 accumulate; dinv pre-scale fused into the relu; pad nodes have
     dinv=0 so their table rows are exactly zero)
  2. AllGather enc shards -> full bf16 table [50176, 128] in DRAM
  3+4. message passing + decoder.  Destination nodes are ordered per core by
     (even-src count, odd-src count); dst tile t (128 nodes) gathers its
     in-edges as pair rows: slot g*128+d lands at partition d, stripe g of
     a [128, G, 256] buffer; the wanted node is the [0:128] (even src) or
     [128:256] (odd src) half of the 256-elem pair row, so per-tile streams
     are split by source parity.  Pad slots point at a guaranteed-zero pair.
     A DVE pairwise tree over stripes (reading the proper half-columns)
     yields agg[d, f] f32; dinv[dst] post-scale rides the f32->bf16 copy;
     a PE identity-matmul transposes agg into decoder lhsT layout;
     out = sigmoid(aggT.T @ W_dec + b_dec_eff), written bf16 and upcast on
     the host, which also un-permutes rows.
"""
import sys

if "/opt/trn_rl_repo" not in sys.path:
    sys.path.insert(0, "/opt/trn_rl_repo")

import numpy as np
import ml_dtypes

import concourse.bacc as bacc
import concourse.bass as bass
import concourse.mybir as mybir
import concourse.tile as tile
from concourse.bass_utils import run_bass_kernel_spmd
from concourse.masks import make_identity

NCORES = 8
N = 50000
IN_SIZE = 512
HID = 128
P = 128
REAL = 6250             # real nodes per core
NCN = 6272              # padded nodes per core = 49 * 128
NTILES = NCN // P       # 49
NPAD = NCORES * NCN     # 50176
NPAIR = NPAD // 2       # 25088 pair rows (int16-addressable)
ZROW = 3125             # zero pair row (core-0 pad nodes 6250, 6251)
CALLW = 896             # max slots per dma_gather call (ring cap ~1024)
CAPG = 100              # max stripes per gather group (SBUF budget)
SHARED_AG = False       # addr_space="Shared" for the AllGather output
SPKT = True             # single_packet for dma_gather
NQUEUES = 4
CHUNK_AG = True         # split AllGather into 4 chunks overlapped w/ encoder
CS = [0, 1536, 3072, 4608, 6272]   # local row chunk bounds (tile-aligned)

_cache = {}


def _wrap_idx(arr):
    """int16 index array -> [128, len/16] wrapped layout: slot i at
    [i % 16, i // 16], replicated for the 8 gpsimd cores' partition groups."""
    a = np.asarray(arr, np.int16)
    assert len(a) % 16 == 0
    w = a.reshape(-1, 16).T
    return np.ascontiguousarray(np.tile(w, (8, 1)))


def _build_schedule(edge_index):
    src0 = np.asarray(edge_index[0], np.int64)
    dst0 = np.asarray(edge_index[1], np.int64)
    loops = np.arange(N, dtype=np.int64)
    src0 = np.concatenate([src0, loops])
    dst0 = np.concatenate([dst0, loops])
    # renumber: real node n -> (n//REAL)*NCN + n%REAL  (pads at shard tails)
    dst = (dst0 // REAL) * NCN + dst0 % REAL
    if CHUNK_AG:
        # chunk-major table positions: each partial AllGather writes a
        # contiguous slab [8*CS[q], 8*CS[q+1])
        cs_, ls_ = src0 // REAL, src0 % REAL
        q_ = np.searchsorted(CS, ls_, side="right") - 1
        csa = np.asarray(CS, np.int64)
        cl_ = csa[q_ + 1] - csa[q_]
        src = 8 * csa[q_] + cs_ * cl_ + (ls_ - csa[q_])
    else:
        src = (src0 // REAL) * NCN + src0 % REAL

    deg = np.bincount(dst, minlength=NPAD)
    dinv = np.zeros(NPAD, np.float32)
    nz = deg > 0
    dinv[nz] = (1.0 / np.sqrt(deg[nz].astype(np.float64))).astype(np.float32)

    core = dst // NCN
    local = dst % NCN
    par = (src % 2).astype(np.int64)     # source-parity stream split

    # per (core, local, parity) in-edge counts
    cnt = np.bincount((core * NCN + local) * 2 + par,
                      minlength=NPAD * 2).reshape(NCORES, NCN, 2)

    # per-core node order: sort locals by (even cnt, odd cnt)
    sig = np.zeros((NCORES, NCN), np.int64)      # sig[c, j] = local node
    pos = np.zeros((NCORES, NCN), np.int64)      # pos[c, local] = j
    for c in range(NCORES):
        s = np.lexsort((cnt[c, :, 1], cnt[c, :, 0]))
        sig[c] = s
        pos[c, s] = np.arange(NCN)

    # per-(tile, parity) stripe counts: max over cores and in-tile nodes
    cnt_sorted = np.take_along_axis(cnt, sig[:, :, None], axis=1)
    G = cnt_sorted.reshape(NCORES, NTILES, P, 2).max(axis=2).max(axis=0)
    G = G.astype(np.int64)                       # [NTILES, 2]
    odd = (G[:, 0] + G[:, 1]) % 2 == 1
    G[odd, 1] += 1                               # per-tile stripe total even

    # greedy grouping of consecutive tiles under the SBUF stripe cap
    Gtot = G.sum(axis=1)
    groups = []
    a = 0
    while a < NTILES:
        b = a + 1
        s = Gtot[a]
        while b < NTILES and s + Gtot[b] <= CAPG:
            s += Gtot[b]
            b += 1
        groups.append((a, b))
        a = b

    # stream layout: group-major; within group: parity-0 blocks of its
    # tiles, then parity-1 blocks.  All offsets in slots.
    boff = np.zeros((NTILES, 2), np.int64)
    gmeta = []
    run = 0
    for (a, b) in groups:
        gs = run
        for h in (0, 1):
            for t in range(a, b):
                boff[t, h] = run
                run += P * int(G[t, h])
        gmeta.append({"gs": gs, "ge": run, "tiles": (a, b)})
    TOT = run
    assert TOT % P == 0

    # slot stream: rank of each edge within its (core, local, parity) group
    order = np.lexsort((src, par, local, core))
    o_src, o_core, o_local, o_par = (src[order], core[order],
                                     local[order], par[order])
    okey = (o_core * NCN + o_local) * 2 + o_par
    gstart = np.concatenate([[0], np.cumsum(np.bincount(
        okey, minlength=NPAD * 2))])[:-1]
    grank = np.arange(len(okey)) - gstart[okey]

    j = pos[o_core, o_local]
    t_ = j // P
    d_ = j % P
    slot = boff[t_, o_par] + grank * P + d_
    # pad slots round-robin over all 88 zero pair rows (8 shards x 11 pairs)
    # so pad gathers don't serialize on one hot HBM row
    if CHUNK_AG:
        cll = CS[-1] - CS[-2]
        zp = np.array([(8 * CS[-2] + c * cll + (REAL - CS[-2])) // 2 + k
                       for c in range(NCORES) for k in range(11)], np.int64)
    else:
        zp = np.array([(c * NCN + REAL) // 2 + k
                       for c in range(NCORES) for k in range(11)], np.int64)
    gidx = np.tile(zp[np.arange(TOT) % len(zp)], (NCORES, 1))
    gidx[o_core, slot] = o_src // 2
    assert gidx.max() < NPAIR and gidx.min() >= 0

    return {
        "dinv": dinv, "G": G, "boff": boff, "gmeta": gmeta, "TOT": TOT,
        "gidx": gidx, "sig": sig,
    }


def _build_nc(sched, repeat=1, rep_enc=1, rep_ag=1, rep_mp=1,
              mp_gather=True, mp_reduce=True):
    G, boff, gmeta, TOT = sched["G"], sched["boff"], sched["gmeta"], sched["TOT"]

    nc = bacc.Bacc("TRN2", target_bir_lowering=False, debug=False,
                   num_devices=NCORES, num_swdge_queues=NQUEUES)
    f32, bf16, i16 = mybir.dt.float32, mybir.dt.bfloat16, mybir.dt.int16

    xT = nc.dram_tensor("xT", [IN_SIZE, NCN], bf16, kind="ExternalInput")
    w_enc = nc.dram_tensor("w_enc", [IN_SIZE, HID], bf16, kind="ExternalInput")
    b_enc = nc.dram_tensor("b_enc", [1, HID], bf16, kind="ExternalInput")
    w_dec = nc.dram_tensor("w_dec", [HID, IN_SIZE], bf16, kind="ExternalInput")
    b_dec = nc.dram_tensor("b_dec", [1, IN_SIZE], bf16, kind="ExternalInput")
    dinv_e = nc.dram_tensor("dinv_e", [P, NTILES], f32, kind="ExternalInput")
    dinvs = nc.dram_tensor("dinvs", [P, NTILES], f32, kind="ExternalInput")
    g_d = nc.dram_tensor("gidx", [P, TOT // 16], i16, kind="ExternalInput")
    out = nc.dram_tensor("out", [NCN, IN_SIZE], bf16, kind="ExternalOutput")

    ACCW = max(int(G[t, 0] + G[t, 1] + 1) // 2 + 1 for t in range(NTILES))

    with tile.TileContext(nc) as tc:
        with (
            tc.tile_pool(name="const", bufs=1) as cp,
            tc.tile_pool(name="dram", bufs=1, space="DRAM") as dram,
            tc.tile_pool(name="psum", bufs=2, space="PSUM") as pp,
        ):
            # ---- constants ----
            ones = cp.tile([1, P], bf16)
            nc.vector.memset(ones[:], 1.0)
            ident = cp.tile([P, P], bf16)
            make_identity(nc, ident)
            benc_sb = cp.tile([1, HID], bf16)
            nc.sync.dma_start(benc_sb[:], b_enc[:])
            bdec_sb = cp.tile([1, IN_SIZE], bf16)
            nc.sync.dma_start(bdec_sb[:], b_dec[:])
            wdec_sb = cp.tile([HID, IN_SIZE], bf16)
            nc.sync.dma_start(wdec_sb[:], w_dec[:])
            dinv_e_sb = cp.tile([P, NTILES], f32)
            nc.sync.dma_start(dinv_e_sb[:], dinv_e[:])
            dinvs_sb = cp.tile([P, NTILES], f32)
            nc.sync.dma_start(dinvs_sb[:], dinvs[:])

            enc_loc = dram.tile([NCN, HID], bf16)
            # Shared DRAM enforces a single writer instruction, so timing
            # builds (repeat>1) fall back to Local; reps then serialize on
            # the table, keeping the (t3-t1)/2 methodology honest.
            enc_all = dram.tile([NPAD, HID], bf16,
                                addr_space="Shared"
                                if (SHARED_AG and repeat == 1 and rep_ag == 1)
                                else "Local")

            for _rep in range(repeat):
                # ---- phase 1: encoder ----
                for _renc in range(rep_enc):
                  with tc.tile_pool(name="ph1", bufs=1) as p1, \
                     tc.tile_pool(name="ph1db", bufs=3) as p1db:
                    wenc_sb = p1.tile([P, 4, HID], bf16)
                    for k in range(4):
                        nc.sync.dma_start(wenc_sb[:, k, :],
                                          w_enc[k * P:(k + 1) * P, :])
                    xt_sb = p1.tile([P, 4, NCN], bf16)
                    for k in range(4):
                        nc.sync.dma_start(xt_sb[:, k, :],
                                          xT[k * P:(k + 1) * P, :])
                    for t in range(NTILES):
                        ps = pp.tile([P, HID], f32, tag="ps_enc")
                        nc.tensor.matmul(ps[:], ones[:1, :], benc_sb[:1, :],
                                         start=True, stop=False)
                        for k in range(4):
                            nc.tensor.matmul(
                                ps[:], xt_sb[:, k, t * P:(t + 1) * P],
                                wenc_sb[:, k, :], start=False, stop=(k == 3))
                        enc_t = p1db.tile([P, HID], bf16, tag="enc_t")
                        nc.scalar.activation(enc_t[:], ps[:],
                                             mybir.ActivationFunctionType.Relu,
                                             scale=dinv_e_sb[:, t:t + 1])
                        nc.sync.dma_start(enc_loc[t * P:(t + 1) * P, :], enc_t[:])
                        if CHUNK_AG and (t + 1) * P in CS:
                            qq = CS.index((t + 1) * P) - 1
                            r0, r1 = CS[qq], CS[qq + 1]
                            nc.gpsimd.collective_compute(
                                "AllGather", mybir.AluOpType.bypass,
                                replica_groups=[list(range(NCORES))],
                                ins=[enc_loc[r0:r1, :]],
                                outs=[enc_all[8 * r0:8 * r1, :]],
                            )

                # ---- phase 2: allgather ----
                for _rag in range(0 if CHUNK_AG else rep_ag):
                    nc.gpsimd.collective_compute(
                        "AllGather", mybir.AluOpType.bypass,
                        replica_groups=[list(range(NCORES))],
                        ins=[enc_loc.opt()], outs=[enc_all.opt()],
                    )

                # ---- phase 3+4: pair-row gather, reduce, decode ----
                pair_tbl = enc_all[:].rearrange("(a b) h -> a (b h)", b=2)
                for _rmp in range(rep_mp):
                  with tc.tile_pool(name="ph3", bufs=1) as p3, \
                     tc.tile_pool(name="gb", bufs=2) as gbp, \
                     tc.tile_pool(name="acc", bufs=2) as accp, \
                     tc.tile_pool(name="ph4", bufs=3) as p4:
                    g_sb = p3.tile([P, TOT // 16], i16)
                    nc.sync.dma_start(g_sb[:], g_d[:])

                    q = [0]
                    for gm in gmeta:
                        gs, ge = gm["gs"], gm["ge"]
                        buf = gbp.tile([P, CAPG, 2 * HID], bf16, tag="gbuf")
                        if not mp_gather:
                            nc.vector.memset(buf[:, :(ge - gs) // P, :], 0.0)
                        a = gs
                        while mp_gather and a < ge:
                            nw = min(CALLW, ge - a)
                            so = (a - gs) // P
                            nc.gpsimd.dma_gather(
                                buf[:, so:so + nw // P, :], pair_tbl,
                                g_sb[:, a // 16:(a + nw) // 16],
                                nw, nw, 2 * HID,
                                transpose=False,
                                queue_num=q[0], single_packet=SPKT)
                            q[0] = (q[0] + 1) % NQUEUES
                            a += nw

                        ta, tb = gm["tiles"]
                        for t in range(ta, tb) if mp_reduce else ():
                            s0 = (boff[t, 0] - gs) // P
                            n0 = int(G[t, 0])
                            s1 = (boff[t, 1] - gs) // P
                            n1 = int(G[t, 1])
                            # column range of the wanted half per parity
                            c0 = slice(0, HID)
                            c1 = slice(HID, 2 * HID)
                            acc = accp.tile([P, ACCW, HID], f32, tag="acc")
                            w = 0
                            k0, k1 = n0 // 2, n1 // 2
                            if k0:
                                nc.vector.tensor_add(
                                    acc[:, w:w + k0, :],
                                    buf[:, s0:s0 + k0, c0],
                                    buf[:, s0 + k0:s0 + 2 * k0, c0])
                                w += k0
                            if k1:
                                nc.vector.tensor_add(
                                    acc[:, w:w + k1, :],
                                    buf[:, s1:s1 + k1, c1],
                                    buf[:, s1 + k1:s1 + 2 * k1, c1])
                                w += k1
                            if n0 % 2 and n1 % 2:
                                nc.vector.tensor_add(
                                    acc[:, w:w + 1, :],
                                    buf[:, s0 + 2 * k0:s0 + 2 * k0 + 1, c0],
                                    buf[:, s1 + 2 * k1:s1 + 2 * k1 + 1, c1])
                                w += 1
                            else:
                                assert n0 % 2 == 0 and n1 % 2 == 0, (n0, n1)
                            if w == 0:
                                nc.vector.memset(acc[:, 0:1, :], 0.0)
                                w = 1
                            while w > 1:
                                k = w // 2
                                nc.vector.tensor_add(acc[:, :k, :],
                                                     acc[:, :k, :],
                                                     acc[:, k:2 * k, :])
                                if w % 2:
                                    nc.vector.tensor_add(
                                        acc[:, :1, :], acc[:, :1, :],
                                        acc[:, 2 * k:2 * k + 1, :])
                                w = k

                            agg_bf = p4.tile([P, P], bf16, tag="agg_bf")
                            nc.scalar.activation(
                                agg_bf[:], acc[:, 0, :],
                                mybir.ActivationFunctionType.Copy,
                                scale=dinvs_sb[:, t:t + 1])
                            psT = pp.tile([P, P], f32, tag="psT")
                            nc.tensor.matmul(psT[:], agg_bf[:], ident[:],
                                             start=True, stop=True)
                            aggT = p4.tile([P, P], bf16, tag="aggT")
                            nc.scalar.activation(
                                aggT[:], psT[:],
                                mybir.ActivationFunctionType.Copy)
                            ps = pp.tile([P, IN_SIZE], f32, tag="ps_dec")
                            nc.tensor.matmul(ps[:], ones[:1, :],
                                             bdec_sb[:1, :],
                                             start=True, stop=False)
                            nc.tensor.matmul(ps[:], aggT[:], wdec_sb[:],
                                             start=False, stop=True)
                            o_t = p4.tile([P, IN_SIZE], bf16, tag="o_t")
                            nc.scalar.activation(
                                o_t[:], ps[:],
                                mybir.ActivationFunctionType.Sigmoid)
                            nc.sync.dma_start(out[t * P:(t + 1) * P, :],
                                              o_t[:])

    nc.compile()
    return nc


def _prepare(x, W_enc, b_enc, W_dec, b_dec, gcn_bias, edge_index):
    sched = _build_schedule(edge_index)
    dinv, sig = sched["dinv"], sched["sig"]

    x = np.asarray(x, np.float32)
    b_dec_eff = (np.asarray(gcn_bias, np.float32) @
                 np.asarray(W_dec, np.float32) +
                 np.asarray(b_dec, np.float32))

    in_maps = []
    for c in range(NCORES):
        xc = np.zeros((NCN, IN_SIZE), np.float32)
        xc[:REAL] = x[c * REAL:(c + 1) * REAL]
        xT_c = np.ascontiguousarray(xc.T.astype(ml_dtypes.bfloat16))
        dv = dinv[c * NCN:(c + 1) * NCN]
        dinv_e_c = np.ascontiguousarray(
            dv.reshape(NTILES, P).T.astype(np.float32))
        dinvs_c = np.ascontiguousarray(
            dv[sig[c]].reshape(NTILES, P).T.astype(np.float32))
        in_maps.append({
            "xT": xT_c,
            "w_enc": np.asarray(W_enc, np.float32).astype(ml_dtypes.bfloat16),
            "b_enc": np.asarray(b_enc, np.float32).reshape(1, -1)
                       .astype(ml_dtypes.bfloat16),
            "w_dec": np.asarray(W_dec, np.float32).astype(ml_dtypes.bfloat16),
            "b_dec": b_dec_eff.reshape(1, -1).astype(ml_dtypes.bfloat16),
            "dinv_e": dinv_e_c,
            "dinvs": dinvs_c,
            "gidx": _wrap_idx(sched["gidx"][c]),
        })
    return sched, in_maps


def kernel(x, W_enc, b_enc, W_dec, b_dec, gcn_bias, edge_index,
           _profile=False):
    key = hash(np.asarray(edge_index).tobytes())
    sched, in_maps = _prepare(x, W_enc, b_enc, W_dec, b_dec, gcn_bias,
                              edge_index)
    if key in _cache:
        nc = _cache[key]
    else:
        nc = _build_nc(sched)
        _cache[key] = nc

    res = run_bass_kernel_spmd(nc, in_maps, core_ids=list(range(NCORES)),
                               trace=_profile)
    sig = sched["sig"]
    outp = np.empty((N, IN_SIZE), np.float32)
    for c in range(NCORES):
        o = np.asarray(res.results[c]["out"], np.float32)
        mask = sig[c] < REAL
        outp[c * REAL + sig[c][mask]] = o[mask]
    if _profile:
        return outp, res
    return outp



# revision 7
# speedup vs baseline: 7.7193x; 7.7193x over previous
"""GCN autoencoder (encoder -> GCNConv -> decoder) on 8 TRN2 NeuronCores, v4.

Key idea vs the old SBUF-transpose-gather baseline: message passing gathers
512-byte PAIR rows (two adjacent nodes' bf16 features) straight from the
DRAM table with non-transpose dma_gather.  512B descriptors run ~4.7x
faster per descriptor than 256B ones on TRN2's SWDGE path, and pair ids
(node_position // 2 < 25088) fit int16 without splitting the table.

Per core c (nodes sharded 6250 real + 22 zero-pad = 6272 = 49*128 per core):
  1. encoder:  enc[n] = dinv[n] * relu(x[n] @ W_enc + b_enc)  (bf16 matmuls,
     f32 accumulate; dinv pre-scale fused into the relu; pad nodes have
     dinv=0 so their table rows are exactly zero)
  2. AllGather enc shards -> full bf16 table [50176, 128] in DRAM
  3+4. message passing + decoder.  Destination nodes are ordered per core by
     (even-src count, odd-src count); dst tile t (128 nodes) gathers its
     in-edges as pair rows: slot g*128+d lands at partition d, stripe g of
     a [128, G, 256] buffer; the wanted node is the [0:128] (even src) or
     [128:256] (odd src) half of the 256-elem pair row, so per-tile streams
     are split by source parity.  Pad slots point at a guaranteed-zero pair.
     A DVE pairwise tree over stripes (reading the proper half-columns)
     yields agg[d, f] f32; dinv[dst] post-scale rides the f32->bf16 copy;
     a PE identity-matmul transposes agg into decoder lhsT layout;
     out = sigmoid(aggT.T @ W_dec + b_dec_eff), written bf16 and upcast on
     the host, which also un-permutes rows.
"""
import sys

if "/opt/trn_rl_repo" not in sys.path:
    sys.path.insert(0, "/opt/trn_rl_repo")

import numpy as np
import ml_dtypes

import concourse.bacc as bacc
import concourse.bass as bass
import concourse.mybir as mybir
import concourse.tile as tile
from concourse.bass_utils import run_bass_kernel_spmd
from concourse.masks import make_identity

NCORES = 8
N = 50000
IN_SIZE = 512
HID = 128
P = 128
REAL = 6250             # real nodes per core
NCN = 6272              # padded nodes per core = 49 * 128
NTILES = NCN // P       # 49
NPAD = NCORES * NCN     # 50176
NPAIR = NPAD // 2       # 25088 pair rows (int16-addressable)
ZROW = 3125             # zero pair row (core-0 pad nodes 6250, 6251)
CALLW = 896             # max slots per dma_gather call (ring cap ~1024)
CAPG = 100              # max stripes per gather group (SBUF budget)
SHARED_AG = False       # addr_space="Shared" for the AllGather output
SPKT = True             # single_packet for dma_gather
NQUEUES = 4
CHUNK_AG = True         # split AllGather into 4 chunks overlapped w/ encoder
CS = [0, 1536, 3072, 4608, 6272]   # local row chunk bounds (tile-aligned)
TBL_FP8 = True          # fp8 (e4m3) gather table: 256B pair rows
SNAKE = True            # snake secondary sort: less odd-parity padding

_cache = {}


def _wrap_idx(arr):
    """int16 index array -> [128, len/16] wrapped layout: slot i at
    [i % 16, i // 16], replicated for the 8 gpsimd cores' partition groups."""
    a = np.asarray(arr, np.int16)
    assert len(a) % 16 == 0
    w = a.reshape(-1, 16).T
    return np.ascontiguousarray(np.tile(w, (8, 1)))


def _build_schedule(edge_index):
    src0 = np.asarray(edge_index[0], np.int64)
    dst0 = np.asarray(edge_index[1], np.int64)
    loops = np.arange(N, dtype=np.int64)
    src0 = np.concatenate([src0, loops])
    dst0 = np.concatenate([dst0, loops])
    # renumber: real node n -> (n//REAL)*NCN + n%REAL  (pads at shard tails)
    dst = (dst0 // REAL) * NCN + dst0 % REAL
    if CHUNK_AG:
        # chunk-major table positions: each partial AllGather writes a
        # contiguous slab [8*CS[q], 8*CS[q+1])
        cs_, ls_ = src0 // REAL, src0 % REAL
        q_ = np.searchsorted(CS, ls_, side="right") - 1
        csa = np.asarray(CS, np.int64)
        cl_ = csa[q_ + 1] - csa[q_]
        src = 8 * csa[q_] + cs_ * cl_ + (ls_ - csa[q_])
    else:
        src = (src0 // REAL) * NCN + src0 % REAL

    deg = np.bincount(dst, minlength=NPAD)
    dinv = np.zeros(NPAD, np.float32)
    nz = deg > 0
    dinv[nz] = (1.0 / np.sqrt(deg[nz].astype(np.float64))).astype(np.float32)

    core = dst // NCN
    local = dst % NCN
    par = (src % 2).astype(np.int64)     # source-parity stream split

    # per (core, local, parity) in-edge counts
    cnt = np.bincount((core * NCN + local) * 2 + par,
                      minlength=NPAD * 2).reshape(NCORES, NCN, 2)

    # per-core node order: sort locals by (even cnt, odd cnt); snake flips
    # the odd-cnt direction on alternate even-cnt runs so tile boundaries
    # don't jump from max-odd back to min-odd
    sig = np.zeros((NCORES, NCN), np.int64)      # sig[c, j] = local node
    pos = np.zeros((NCORES, NCN), np.int64)      # pos[c, local] = j
    for c in range(NCORES):
        e, o = cnt[c, :, 0], cnt[c, :, 1]
        key2 = np.where(e % 2 == 0, o, o.max() - o) if SNAKE else o
        s = np.lexsort((key2, e))
        sig[c] = s
        pos[c, s] = np.arange(NCN)

    # per-(tile, parity) stripe counts: max over cores and in-tile nodes
    cnt_sorted = np.take_along_axis(cnt, sig[:, :, None], axis=1)
    G = cnt_sorted.reshape(NCORES, NTILES, P, 2).max(axis=2).max(axis=0)
    G = G.astype(np.int64)                       # [NTILES, 2]
    odd = (G[:, 0] + G[:, 1]) % 2 == 1
    G[odd, 1] += 1                               # per-tile stripe total even

    # greedy grouping of consecutive tiles under the SBUF stripe cap
    Gtot = G.sum(axis=1)
    groups = []
    a = 0
    while a < NTILES:
        b = a + 1
        s = Gtot[a]
        while b < NTILES and s + Gtot[b] <= CAPG:
            s += Gtot[b]
            b += 1
        groups.append((a, b))
        a = b

    # stream layout: group-major; within group: parity-0 blocks of its
    # tiles, then parity-1 blocks.  All offsets in slots.
    boff = np.zeros((NTILES, 2), np.int64)
    gmeta = []
    run = 0
    for (a, b) in groups:
        gs = run
        for h in (0, 1):
            for t in range(a, b):
                boff[t, h] = run
                run += P * int(G[t, h])
        gmeta.append({"gs": gs, "ge": run, "tiles": (a, b)})
    TOT = run
    assert TOT % P == 0

    # slot stream: rank of each edge within its (core, local, parity) group
    order = np.lexsort((src, par, local, core))
    o_src, o_core, o_local, o_par = (src[order], core[order],
                                     local[order], par[order])
    okey = (o_core * NCN + o_local) * 2 + o_par
    gstart = np.concatenate([[0], np.cumsum(np.bincount(
        okey, minlength=NPAD * 2))])[:-1]
    grank = np.arange(len(okey)) - gstart[okey]

    j = pos[o_core, o_local]
    t_ = j // P
    d_ = j % P
    slot = boff[t_, o_par] + grank * P + d_
    # pad slots round-robin over all 88 zero pair rows (8 shards x 11 pairs)
    # so pad gathers don't serialize on one hot HBM row
    if CHUNK_AG:
        cll = CS[-1] - CS[-2]
        zp = np.array([(8 * CS[-2] + c * cll + (REAL - CS[-2])) // 2 + k
                       for c in range(NCORES) for k in range(11)], np.int64)
    else:
        zp = np.array([(c * NCN + REAL) // 2 + k
                       for c in range(NCORES) for k in range(11)], np.int64)
    gidx = np.tile(zp[np.arange(TOT) % len(zp)], (NCORES, 1))
    gidx[o_core, slot] = o_src // 2
    assert gidx.max() < NPAIR and gidx.min() >= 0

    return {
        "dinv": dinv, "G": G, "boff": boff, "gmeta": gmeta, "TOT": TOT,
        "gidx": gidx, "sig": sig,
    }


def _build_nc(sched, repeat=1, rep_enc=1, rep_ag=1, rep_mp=1,
              mp_gather=True, mp_reduce=True):
    G, boff, gmeta, TOT = sched["G"], sched["boff"], sched["gmeta"], sched["TOT"]

    nc = bacc.Bacc("TRN2", target_bir_lowering=False, debug=False,
                   num_devices=NCORES, num_swdge_queues=NQUEUES)
    f32, bf16, i16 = mybir.dt.float32, mybir.dt.bfloat16, mybir.dt.int16
    tbl_dt = mybir.dt.float8e4 if TBL_FP8 else bf16

    xT = nc.dram_tensor("xT", [IN_SIZE, NCN], bf16, kind="ExternalInput")
    w_enc = nc.dram_tensor("w_enc", [IN_SIZE, HID], bf16, kind="ExternalInput")
    b_enc = nc.dram_tensor("b_enc", [1, HID], bf16, kind="ExternalInput")
    w_dec = nc.dram_tensor("w_dec", [HID, IN_SIZE], bf16, kind="ExternalInput")
    b_dec = nc.dram_tensor("b_dec", [1, IN_SIZE], bf16, kind="ExternalInput")
    dinv_e = nc.dram_tensor("dinv_e", [P, NTILES], f32, kind="ExternalInput")
    dinvs = nc.dram_tensor("dinvs", [P, NTILES], f32, kind="ExternalInput")
    g_d = nc.dram_tensor("gidx", [P, TOT // 16], i16, kind="ExternalInput")
    out = nc.dram_tensor("out", [NCN, IN_SIZE], bf16, kind="ExternalOutput")

    ACCW = max(int(G[t, 0] + G[t, 1] + 1) // 2 + 1 for t in range(NTILES))

    with tile.TileContext(nc) as tc:
        with (
            tc.tile_pool(name="const", bufs=1) as cp,
            tc.tile_pool(name="dram", bufs=1, space="DRAM") as dram,
            tc.tile_pool(name="psum", bufs=2, space="PSUM") as pp,
        ):
            # ---- constants ----
            ones = cp.tile([1, P], bf16)
            nc.vector.memset(ones[:], 1.0)
            ident = cp.tile([P, P], bf16)
            make_identity(nc, ident)
            benc_sb = cp.tile([1, HID], bf16)
            nc.sync.dma_start(benc_sb[:], b_enc[:])
            bdec_sb = cp.tile([1, IN_SIZE], bf16)
            nc.sync.dma_start(bdec_sb[:], b_dec[:])
            wdec_sb = cp.tile([HID, IN_SIZE], bf16)
            nc.sync.dma_start(wdec_sb[:], w_dec[:])
            dinv_e_sb = cp.tile([P, NTILES], f32)
            nc.sync.dma_start(dinv_e_sb[:], dinv_e[:])
            dinvs_sb = cp.tile([P, NTILES], f32)
            nc.sync.dma_start(dinvs_sb[:], dinvs[:])

            enc_loc = dram.tile([NCN, HID], tbl_dt)
            # Shared DRAM enforces a single writer instruction, so timing
            # builds (repeat>1) fall back to Local; reps then serialize on
            # the table, keeping the (t3-t1)/2 methodology honest.
            enc_all = dram.tile([NPAD, HID], tbl_dt,
                                addr_space="Shared"
                                if (SHARED_AG and repeat == 1 and rep_ag == 1)
                                else "Local")

            for _rep in range(repeat):
                # ---- phase 1: encoder ----
                for _renc in range(rep_enc):
                  with tc.tile_pool(name="ph1", bufs=1) as p1, \
                     tc.tile_pool(name="ph1db", bufs=3) as p1db:
                    wenc_sb = p1.tile([P, 4, HID], bf16)
                    for k in range(4):
                        nc.sync.dma_start(wenc_sb[:, k, :],
                                          w_enc[k * P:(k + 1) * P, :])
                    xt_sb = p1.tile([P, 4, NCN], bf16)
                    for k in range(4):
                        nc.sync.dma_start(xt_sb[:, k, :],
                                          xT[k * P:(k + 1) * P, :])
                    for t in range(NTILES):
                        ps = pp.tile([P, HID], f32, tag="ps_enc")
                        nc.tensor.matmul(ps[:], ones[:1, :], benc_sb[:1, :],
                                         start=True, stop=False)
                        for k in range(4):
                            nc.tensor.matmul(
                                ps[:], xt_sb[:, k, t * P:(t + 1) * P],
                                wenc_sb[:, k, :], start=False, stop=(k == 3))
                        enc_t = p1db.tile([P, HID], tbl_dt, tag="enc_t")
                        nc.scalar.activation(enc_t[:], ps[:],
                                             mybir.ActivationFunctionType.Relu,
                                             scale=dinv_e_sb[:, t:t + 1])
                        nc.sync.dma_start(enc_loc[t * P:(t + 1) * P, :], enc_t[:])
                        if CHUNK_AG and (t + 1) * P in CS:
                            qq = CS.index((t + 1) * P) - 1
                            r0, r1 = CS[qq], CS[qq + 1]
                            nc.gpsimd.collective_compute(
                                "AllGather", mybir.AluOpType.bypass,
                                replica_groups=[list(range(NCORES))],
                                ins=[enc_loc[r0:r1, :]],
                                outs=[enc_all[8 * r0:8 * r1, :]],
                            )

                # ---- phase 2: allgather ----
                for _rag in range(0 if CHUNK_AG else rep_ag):
                    nc.gpsimd.collective_compute(
                        "AllGather", mybir.AluOpType.bypass,
                        replica_groups=[list(range(NCORES))],
                        ins=[enc_loc.opt()], outs=[enc_all.opt()],
                    )

                # ---- phase 3+4: pair-row gather, reduce, decode ----
                pair_tbl = enc_all[:].rearrange("(a b) h -> a (b h)", b=2)
                for _rmp in range(rep_mp):
                  with tc.tile_pool(name="ph3", bufs=1) as p3, \
                     tc.tile_pool(name="gb", bufs=2) as gbp, \
                     tc.tile_pool(name="acc", bufs=2) as accp, \
                     tc.tile_pool(name="ph4", bufs=3) as p4:
                    g_sb = p3.tile([P, TOT // 16], i16)
                    nc.sync.dma_start(g_sb[:], g_d[:])

                    q = [0]
                    for gm in gmeta:
                        gs, ge = gm["gs"], gm["ge"]
                        buf = gbp.tile([P, CAPG, 2 * HID], tbl_dt, tag="gbuf")
                        if not mp_gather:
                            nc.vector.memset(buf[:, :(ge - gs) // P, :], 0.0)
                        a = gs
                        while mp_gather and a < ge:
                            nw = min(CALLW, ge - a)
                            so = (a - gs) // P
                            nc.gpsimd.dma_gather(
                                buf[:, so:so + nw // P, :], pair_tbl,
                                g_sb[:, a // 16:(a + nw) // 16],
                                nw, nw, 2 * HID,
                                transpose=False,
                                queue_num=q[0], single_packet=SPKT)
                            q[0] = (q[0] + 1) % NQUEUES
                            a += nw

                        ta, tb = gm["tiles"]
                        for t in range(ta, tb) if mp_reduce else ():
                            s0 = (boff[t, 0] - gs) // P
                            n0 = int(G[t, 0])
                            s1 = (boff[t, 1] - gs) // P
                            n1 = int(G[t, 1])
                            # column range of the wanted half per parity
                            c0 = slice(0, HID)
                            c1 = slice(HID, 2 * HID)
                            acc = accp.tile([P, ACCW, HID], f32, tag="acc")
                            w = 0
                            k0, k1 = n0 // 2, n1 // 2
                            if k0:
                                nc.vector.tensor_add(
                                    acc[:, w:w + k0, :],
                                    buf[:, s0:s0 + k0, c0],
                                    buf[:, s0 + k0:s0 + 2 * k0, c0])
                                w += k0
                            if k1:
                                nc.vector.tensor_add(
                                    acc[:, w:w + k1, :],
                                    buf[:, s1:s1 + k1, c1],
                                    buf[:, s1 + k1:s1 + 2 * k1, c1])
                                w += k1
                            if n0 % 2 and n1 % 2:
                                nc.vector.tensor_add(
                                    acc[:, w:w + 1, :],
                                    buf[:, s0 + 2 * k0:s0 + 2 * k0 + 1, c0],
                                    buf[:, s1 + 2 * k1:s1 + 2 * k1 + 1, c1])
                                w += 1
                            else:
                                assert n0 % 2 == 0 and n1 % 2 == 0, (n0, n1)
                            if w == 0:
                                nc.vector.memset(acc[:, 0:1, :], 0.0)
                                w = 1
                            while w > 1:
                                k = w // 2
                                nc.vector.tensor_add(acc[:, :k, :],
                                                     acc[:, :k, :],
                                                     acc[:, k:2 * k, :])
                                if w % 2:
                                    nc.vector.tensor_add(
                                        acc[:, :1, :], acc[:, :1, :],
                                        acc[:, 2 * k:2 * k + 1, :])
                                w = k

                            agg_bf = p4.tile([P, P], bf16, tag="agg_bf")
                            nc.scalar.activation(
                                agg_bf[:], acc[:, 0, :],
                                mybir.ActivationFunctionType.Copy,
                                scale=dinvs_sb[:, t:t + 1])
                            psT = pp.tile([P, P], f32, tag="psT")
                            nc.tensor.matmul(psT[:], agg_bf[:], ident[:],
                                             start=True, stop=True)
                            aggT = p4.tile([P, P], bf16, tag="aggT")
                            nc.scalar.activation(
                                aggT[:], psT[:],
                                mybir.ActivationFunctionType.Copy)
                            ps = pp.tile([P, IN_SIZE], f32, tag="ps_dec")
                            nc.tensor.matmul(ps[:], ones[:1, :],
                                             bdec_sb[:1, :],
                                             start=True, stop=False)
                            nc.tensor.matmul(ps[:], aggT[:], wdec_sb[:],
                                             start=False, stop=True)
                            o_t = p4.tile([P, IN_SIZE], bf16, tag="o_t")
                            nc.scalar.activation(
                                o_t[:], ps[:],
                                mybir.ActivationFunctionType.Sigmoid)
                            nc.sync.dma_start(out[t * P:(t + 1) * P, :],
                                              o_t[:])

    nc.compile()
    return nc


def _prepare(x, W_enc, b_enc, W_dec, b_dec, gcn_bias, edge_index):
    sched = _build_schedule(edge_index)
    dinv, sig = sched["dinv"], sched["sig"]

    x = np.asarray(x, np.float32)
    b_dec_eff = (np.asarray(gcn_bias, np.float32) @
                 np.asarray(W_dec, np.float32) +
                 np.asarray(b_dec, np.float32))

    in_maps = []
    for c in range(NCORES):
        xc = np.zeros((NCN, IN_SIZE), np.float32)
        xc[:REAL] = x[c * REAL:(c + 1) * REAL]
        xT_c = np.ascontiguousarray(xc.T.astype(ml_dtypes.bfloat16))
        dv = dinv[c * NCN:(c + 1) * NCN]
        dinv_e_c = np.ascontiguousarray(
            dv.reshape(NTILES, P).T.astype(np.float32))
        dinvs_c = np.ascontiguousarray(
            dv[sig[c]].reshape(NTILES, P).T.astype(np.float32))
        in_maps.append({
            "xT": xT_c,
            "w_enc": np.asarray(W_enc, np.float32).astype(ml_dtypes.bfloat16),
            "b_enc": np.asarray(b_enc, np.float32).reshape(1, -1)
                       .astype(ml_dtypes.bfloat16),
            "w_dec": np.asarray(W_dec, np.float32).astype(ml_dtypes.bfloat16),
            "b_dec": b_dec_eff.reshape(1, -1).astype(ml_dtypes.bfloat16),
            "dinv_e": dinv_e_c,
            "dinvs": dinvs_c,
            "gidx": _wrap_idx(sched["gidx"][c]),
        })
    return sched, in_maps


def kernel(x, W_enc, b_enc, W_dec, b_dec, gcn_bias, edge_index,
           _profile=False):
    key = hash(np.asarray(edge_index).tobytes())
    sched, in_maps = _prepare(x, W_enc, b_enc, W_dec, b_dec, gcn_bias,
                              edge_index)
    if key in _cache:
        nc = _cache[key]
    else:
        nc = _build_nc(sched)
        _cache[key] = nc

    res = run_bass_kernel_spmd(nc, in_maps, core_ids=list(range(NCORES)),
                               trace=_profile)
    sig = sched["sig"]
    outp = np.empty((N, IN_SIZE), np.float32)
    for c in range(NCORES):
        o = np.asarray(res.results[c]["out"], np.float32)
        mask = sig[c] < REAL
        outp[c * REAL + sig[c][mask]] = o[mask]
    if _profile:
        return outp, res
    return outp



# revision 13
# speedup vs baseline: 7.7607x; 1.0054x over previous
"""GCN autoencoder (encoder -> GCNConv -> decoder) on 8 TRN2 NeuronCores, v4.

Key idea vs the old SBUF-transpose-gather baseline: message passing gathers
512-byte PAIR rows (two adjacent nodes' bf16 features) straight from the
DRAM table with non-transpose dma_gather.  512B descriptors run ~4.7x
faster per descriptor than 256B ones on TRN2's SWDGE path, and pair ids
(node_position // 2 < 25088) fit int16 without splitting the table.

Per core c (nodes sharded 6250 real + 22 zero-pad = 6272 = 49*128 per core):
  1. encoder:  enc[n] = dinv[n] * relu(x[n] @ W_enc + b_enc)  (bf16 matmuls,
     f32 accumulate; dinv pre-scale fused into the relu; pad nodes have
     dinv=0 so their table rows are exactly zero)
  2. AllGather enc shards -> full bf16 table [50176, 128] in DRAM
  3+4. message passing + decoder.  Destination nodes are ordered per core by
     (even-src count, odd-src count); dst tile t (128 nodes) gathers its
     in-edges as pair rows: slot g*128+d lands at partition d, stripe g of
     a [128, G, 256] buffer; the wanted node is the [0:128] (even src) or
     [128:256] (odd src) half of the 256-elem pair row, so per-tile streams
     are split by source parity.  Pad slots point at a guaranteed-zero pair.
     A DVE pairwise tree over stripes (reading the proper half-columns)
     yields agg[d, f] f32; dinv[dst] post-scale rides the f32->bf16 copy;
     a PE identity-matmul transposes agg into decoder lhsT layout;
     out = sigmoid(aggT.T @ W_dec + b_dec_eff), written bf16 and upcast on
     the host, which also un-permutes rows.
"""
import sys

if "/opt/trn_rl_repo" not in sys.path:
    sys.path.insert(0, "/opt/trn_rl_repo")

import numpy as np
import ml_dtypes

import concourse.bacc as bacc
import concourse.bass as bass
import concourse.mybir as mybir
import concourse.tile as tile
from concourse.bass_utils import run_bass_kernel_spmd
from concourse.masks import make_identity

NCORES = 8
N = 50000
IN_SIZE = 512
HID = 128
P = 128
REAL = 6250             # real nodes per core
NCN = 6272              # padded nodes per core = 49 * 128
NTILES = NCN // P       # 49
NPAD = NCORES * NCN     # 50176
NPAIR = NPAD // 2       # 25088 pair rows (int16-addressable)
ZROW = 3125             # zero pair row (core-0 pad nodes 6250, 6251)
CALLW = 896             # max slots per dma_gather call (ring cap ~1024)
CAPG = 105              # max stripes per gather group (multiple of CALLW/128
                        # so groups split into whole 896-slot gather calls)
SHARED_AG = False       # addr_space="Shared" for the AllGather output
SPKT = True             # single_packet for dma_gather
NQUEUES = 4
CHUNK_AG = True         # split AllGather into 4 chunks overlapped w/ encoder
CS = [0, 2048, 3968, 5632, 6272]   # local row chunk bounds (tile-aligned);
                                   # last chunk small so the AllGather tail
                                   # exposed after the encoder is short
TBL_FP8 = True          # fp8 (e4m3) gather table: 256B pair rows
SNAKE = True            # snake secondary sort: less odd-parity padding

_cache = {}


def _wrap_idx(arr):
    """int16 index array -> [128, len/16] wrapped layout: slot i at
    [i % 16, i // 16], replicated for the 8 gpsimd cores' partition groups."""
    a = np.asarray(arr, np.int16)
    assert len(a) % 16 == 0
    w = a.reshape(-1, 16).T
    return np.ascontiguousarray(np.tile(w, (8, 1)))


def _build_schedule(edge_index):
    src0 = np.asarray(edge_index[0], np.int64)
    dst0 = np.asarray(edge_index[1], np.int64)
    loops = np.arange(N, dtype=np.int64)
    src0 = np.concatenate([src0, loops])
    dst0 = np.concatenate([dst0, loops])
    # renumber: real node n -> (n//REAL)*NCN + n%REAL  (pads at shard tails)
    dst = (dst0 // REAL) * NCN + dst0 % REAL
    if CHUNK_AG:
        # chunk-major table positions: each partial AllGather writes a
        # contiguous slab [8*CS[q], 8*CS[q+1])
        cs_, ls_ = src0 // REAL, src0 % REAL
        q_ = np.searchsorted(CS, ls_, side="right") - 1
        csa = np.asarray(CS, np.int64)
        cl_ = csa[q_ + 1] - csa[q_]
        src = 8 * csa[q_] + cs_ * cl_ + (ls_ - csa[q_])
    else:
        src = (src0 // REAL) * NCN + src0 % REAL

    deg = np.bincount(dst, minlength=NPAD)
    dinv = np.zeros(NPAD, np.float32)
    nz = deg > 0
    dinv[nz] = (1.0 / np.sqrt(deg[nz].astype(np.float64))).astype(np.float32)

    core = dst // NCN
    local = dst % NCN
    par = (src % 2).astype(np.int64)     # source-parity stream split

    # per (core, local, parity) in-edge counts
    cnt = np.bincount((core * NCN + local) * 2 + par,
                      minlength=NPAD * 2).reshape(NCORES, NCN, 2)

    # per-core node order: sort locals by (even cnt, odd cnt); snake flips
    # the odd-cnt direction on alternate even-cnt runs so tile boundaries
    # don't jump from max-odd back to min-odd
    sig = np.zeros((NCORES, NCN), np.int64)      # sig[c, j] = local node
    pos = np.zeros((NCORES, NCN), np.int64)      # pos[c, local] = j
    for c in range(NCORES):
        e, o = cnt[c, :, 0], cnt[c, :, 1]
        key2 = np.where(e % 2 == 0, o, o.max() - o) if SNAKE else o
        s = np.lexsort((key2, e))
        sig[c] = s
        pos[c, s] = np.arange(NCN)

    # per-(tile, parity) stripe counts: max over cores and in-tile nodes
    cnt_sorted = np.take_along_axis(cnt, sig[:, :, None], axis=1)
    G = cnt_sorted.reshape(NCORES, NTILES, P, 2).max(axis=2).max(axis=0)
    G = G.astype(np.int64)                       # [NTILES, 2]
    odd = (G[:, 0] + G[:, 1]) % 2 == 1
    G[odd, 1] += 1                               # per-tile stripe total even

    # greedy grouping of consecutive tiles under the SBUF stripe cap
    Gtot = G.sum(axis=1)
    groups = []
    a = 0
    while a < NTILES:
        b = a + 1
        s = Gtot[a]
        while b < NTILES and s + Gtot[b] <= CAPG:
            s += Gtot[b]
            b += 1
        groups.append((a, b))
        a = b

    # stream layout: group-major; within group: parity-0 blocks of its
    # tiles, then parity-1 blocks.  All offsets in slots.
    boff = np.zeros((NTILES, 2), np.int64)
    gmeta = []
    run = 0
    for (a, b) in groups:
        gs = run
        for h in (0, 1):
            for t in range(a, b):
                boff[t, h] = run
                run += P * int(G[t, h])
        gmeta.append({"gs": gs, "ge": run, "tiles": (a, b)})
    TOT = run
    assert TOT % P == 0

    # slot stream: rank of each edge within its (core, local, parity) group
    order = np.lexsort((src, par, local, core))
    o_src, o_core, o_local, o_par = (src[order], core[order],
                                     local[order], par[order])
    okey = (o_core * NCN + o_local) * 2 + o_par
    gstart = np.concatenate([[0], np.cumsum(np.bincount(
        okey, minlength=NPAD * 2))])[:-1]
    grank = np.arange(len(okey)) - gstart[okey]

    j = pos[o_core, o_local]
    t_ = j // P
    d_ = j % P
    slot = boff[t_, o_par] + grank * P + d_
    # pad slots round-robin over all 88 zero pair rows (8 shards x 11 pairs)
    # so pad gathers don't serialize on one hot HBM row
    if CHUNK_AG:
        cll = CS[-1] - CS[-2]
        zp = np.array([(8 * CS[-2] + c * cll + (REAL - CS[-2])) // 2 + k
                       for c in range(NCORES) for k in range(11)], np.int64)
    else:
        zp = np.array([(c * NCN + REAL) // 2 + k
                       for c in range(NCORES) for k in range(11)], np.int64)
    gidx = np.tile(zp[np.arange(TOT) % len(zp)], (NCORES, 1))
    gidx[o_core, slot] = o_src // 2
    assert gidx.max() < NPAIR and gidx.min() >= 0

    return {
        "dinv": dinv, "G": G, "boff": boff, "gmeta": gmeta, "TOT": TOT,
        "gidx": gidx, "sig": sig,
    }


def _build_nc(sched, repeat=1, rep_enc=1, rep_ag=1, rep_mp=1,
              mp_gather=True, mp_reduce=True, callw=None):
    G, boff, gmeta, TOT = sched["G"], sched["boff"], sched["gmeta"], sched["TOT"]
    cw = callw or CALLW

    nc = bacc.Bacc("TRN2", target_bir_lowering=False, debug=False,
                   num_devices=NCORES, num_swdge_queues=NQUEUES)
    f32, bf16, i16 = mybir.dt.float32, mybir.dt.bfloat16, mybir.dt.int16
    tbl_dt = mybir.dt.float8e4 if TBL_FP8 else bf16

    xT = nc.dram_tensor("xT", [IN_SIZE, NCN], bf16, kind="ExternalInput")
    w_enc = nc.dram_tensor("w_enc", [IN_SIZE, HID], bf16, kind="ExternalInput")
    b_enc = nc.dram_tensor("b_enc", [1, HID], bf16, kind="ExternalInput")
    w_dec = nc.dram_tensor("w_dec", [HID, IN_SIZE], bf16, kind="ExternalInput")
    b_dec = nc.dram_tensor("b_dec", [1, IN_SIZE], bf16, kind="ExternalInput")
    dinv_e = nc.dram_tensor("dinv_e", [P, NTILES], f32, kind="ExternalInput")
    dinvs = nc.dram_tensor("dinvs", [P, NTILES], f32, kind="ExternalInput")
    g_d = nc.dram_tensor("gidx", [P, TOT // 16], i16, kind="ExternalInput")
    out = nc.dram_tensor("out", [NCN, IN_SIZE], bf16, kind="ExternalOutput")

    ACCW = max(int(G[t, 0] + G[t, 1] + 1) // 2 + 1 for t in range(NTILES))

    with tile.TileContext(nc) as tc:
        with (
            tc.tile_pool(name="const", bufs=1) as cp,
            tc.tile_pool(name="dram", bufs=1, space="DRAM") as dram,
            tc.tile_pool(name="psum", bufs=2, space="PSUM") as pp,
        ):
            # ---- constants ----
            ones = cp.tile([1, P], bf16)
            nc.vector.memset(ones[:], 1.0)
            ident = cp.tile([P, P], bf16)
            make_identity(nc, ident)
            benc_sb = cp.tile([1, HID], bf16)
            nc.sync.dma_start(benc_sb[:], b_enc[:])
            bdec_sb = cp.tile([1, IN_SIZE], bf16)
            nc.sync.dma_start(bdec_sb[:], b_dec[:])
            wdec_sb = cp.tile([HID, IN_SIZE], bf16)
            nc.sync.dma_start(wdec_sb[:], w_dec[:])
            dinv_e_sb = cp.tile([P, NTILES], f32)
            nc.sync.dma_start(dinv_e_sb[:], dinv_e[:])
            dinvs_sb = cp.tile([P, NTILES], f32)
            nc.sync.dma_start(dinvs_sb[:], dinvs[:])

            enc_loc = dram.tile([NCN, HID], tbl_dt)
            # Shared DRAM enforces a single writer instruction, so timing
            # builds (repeat>1) fall back to Local; reps then serialize on
            # the table, keeping the (t3-t1)/2 methodology honest.
            enc_all = dram.tile([NPAD, HID], tbl_dt,
                                addr_space="Shared"
                                if (SHARED_AG and repeat == 1 and rep_ag == 1)
                                else "Local")

            for _rep in range(repeat):
              with tc.tile_pool(name="gsb", bufs=1) as gp:
                # index table load rides the scalar-engine HWDGE ring so it
                # overlaps the encoder instead of queuing behind its stores
                g_sb = gp.tile([P, TOT // 16], i16)
                nc.scalar.dma_start(g_sb[:], g_d[:])
                # ---- phase 1: encoder ----
                for _renc in range(rep_enc):
                  with tc.tile_pool(name="ph1", bufs=1) as p1, \
                     tc.tile_pool(name="ph1db", bufs=3) as p1db:
                    wenc_sb = p1.tile([P, 4, HID], bf16)
                    for k in range(4):
                        nc.sync.dma_start(wenc_sb[:, k, :],
                                          w_enc[k * P:(k + 1) * P, :])
                    xt_sb = p1.tile([P, 4, NCN], bf16)
                    for k in range(4):
                        nc.sync.dma_start(xt_sb[:, k, :],
                                          xT[k * P:(k + 1) * P, :])
                    for t in range(NTILES):
                        ps = pp.tile([P, HID], f32, tag="ps_enc")
                        nc.tensor.matmul(ps[:], ones[:1, :], benc_sb[:1, :],
                                         start=True, stop=False)
                        for k in range(4):
                            nc.tensor.matmul(
                                ps[:], xt_sb[:, k, t * P:(t + 1) * P],
                                wenc_sb[:, k, :], start=False, stop=(k == 3))
                        enc_t = p1db.tile([P, HID], tbl_dt, tag="enc_t")
                        nc.scalar.activation(enc_t[:], ps[:],
                                             mybir.ActivationFunctionType.Relu,
                                             scale=dinv_e_sb[:, t:t + 1])
                        nc.sync.dma_start(enc_loc[t * P:(t + 1) * P, :], enc_t[:])
                        if CHUNK_AG and (t + 1) * P in CS:
                            qq = CS.index((t + 1) * P) - 1
                            r0, r1 = CS[qq], CS[qq + 1]
                            nc.gpsimd.collective_compute(
                                "AllGather", mybir.AluOpType.bypass,
                                replica_groups=[list(range(NCORES))],
                                ins=[enc_loc[r0:r1, :]],
                                outs=[enc_all[8 * r0:8 * r1, :]],
                            )

                # ---- phase 2: allgather ----
                for _rag in range(0 if CHUNK_AG else rep_ag):
                    nc.gpsimd.collective_compute(
                        "AllGather", mybir.AluOpType.bypass,
                        replica_groups=[list(range(NCORES))],
                        ins=[enc_loc.opt()], outs=[enc_all.opt()],
                    )

                # ---- phase 3+4: pair-row gather, reduce, decode ----
                pair_tbl = enc_all[:].rearrange("(a b) h -> a (b h)", b=2)
                for _rmp in range(rep_mp):
                  with tc.tile_pool(name="ph3", bufs=1) as p3, \
                     tc.tile_pool(name="gb", bufs=2) as gbp, \
                     tc.tile_pool(name="acc", bufs=2) as accp, \
                     tc.tile_pool(name="ph4", bufs=3) as p4:
                    q = [0]
                    for gm in gmeta:
                        gs, ge = gm["gs"], gm["ge"]
                        buf = gbp.tile([P, CAPG, 2 * HID], tbl_dt, tag="gbuf")
                        if not mp_gather:
                            nc.vector.memset(buf[:, :(ge - gs) // P, :], 0.0)
                        a = gs
                        while mp_gather and a < ge:
                            nw = min(cw, ge - a)
                            so = (a - gs) // P
                            nc.gpsimd.dma_gather(
                                buf[:, so:so + nw // P, :], pair_tbl,
                                g_sb[:, a // 16:(a + nw) // 16],
                                nw, nw, 2 * HID,
                                transpose=False,
                                queue_num=q[0], single_packet=SPKT)
                            q[0] = (q[0] + 1) % NQUEUES
                            a += nw

                        ta, tb = gm["tiles"]
                        for t in range(ta, tb) if mp_reduce else ():
                            s0 = (boff[t, 0] - gs) // P
                            n0 = int(G[t, 0])
                            s1 = (boff[t, 1] - gs) // P
                            n1 = int(G[t, 1])
                            # column range of the wanted half per parity
                            c0 = slice(0, HID)
                            c1 = slice(HID, 2 * HID)
                            acc = accp.tile([P, ACCW, HID], f32, tag="acc")
                            w = 0
                            k0, k1 = n0 // 2, n1 // 2
                            if k0:
                                nc.vector.tensor_add(
                                    acc[:, w:w + k0, :],
                                    buf[:, s0:s0 + k0, c0],
                                    buf[:, s0 + k0:s0 + 2 * k0, c0])
                                w += k0
                            if k1:
                                nc.vector.tensor_add(
                                    acc[:, w:w + k1, :],
                                    buf[:, s1:s1 + k1, c1],
                                    buf[:, s1 + k1:s1 + 2 * k1, c1])
                                w += k1
                            if n0 % 2 and n1 % 2:
                                nc.vector.tensor_add(
                                    acc[:, w:w + 1, :],
                                    buf[:, s0 + 2 * k0:s0 + 2 * k0 + 1, c0],
                                    buf[:, s1 + 2 * k1:s1 + 2 * k1 + 1, c1])
                                w += 1
                            else:
                                assert n0 % 2 == 0 and n1 % 2 == 0, (n0, n1)
                            if w == 0:
                                nc.vector.memset(acc[:, 0:1, :], 0.0)
                                w = 1
                            while w > 1:
                                k = w // 2
                                nc.vector.tensor_add(acc[:, :k, :],
                                                     acc[:, :k, :],
                                                     acc[:, k:2 * k, :])
                                if w % 2:
                                    nc.vector.tensor_add(
                                        acc[:, :1, :], acc[:, :1, :],
                                        acc[:, 2 * k:2 * k + 1, :])
                                w = k

                            agg_bf = p4.tile([P, P], bf16, tag="agg_bf")
                            nc.scalar.activation(
                                agg_bf[:], acc[:, 0, :],
                                mybir.ActivationFunctionType.Copy,
                                scale=dinvs_sb[:, t:t + 1])
                            psT = pp.tile([P, P], f32, tag="psT")
                            nc.tensor.matmul(psT[:], agg_bf[:], ident[:],
                                             start=True, stop=True)
                            aggT = p4.tile([P, P], bf16, tag="aggT")
                            nc.scalar.activation(
                                aggT[:], psT[:],
                                mybir.ActivationFunctionType.Copy)
                            ps = pp.tile([P, IN_SIZE], f32, tag="ps_dec")
                            nc.tensor.matmul(ps[:], ones[:1, :],
                                             bdec_sb[:1, :],
                                             start=True, stop=False)
                            nc.tensor.matmul(ps[:], aggT[:], wdec_sb[:],
                                             start=False, stop=True)
                            o_t = p4.tile([P, IN_SIZE], bf16, tag="o_t")
                            nc.scalar.activation(
                                o_t[:], ps[:],
                                mybir.ActivationFunctionType.Sigmoid)
                            nc.sync.dma_start(out[t * P:(t + 1) * P, :],
                                              o_t[:])

    nc.compile()
    return nc


def _prepare(x, W_enc, b_enc, W_dec, b_dec, gcn_bias, edge_index):
    sched = _build_schedule(edge_index)
    dinv, sig = sched["dinv"], sched["sig"]

    x = np.asarray(x, np.float32)
    b_dec_eff = (np.asarray(gcn_bias, np.float32) @
                 np.asarray(W_dec, np.float32) +
                 np.asarray(b_dec, np.float32))

    in_maps = []
    for c in range(NCORES):
        xc = np.zeros((NCN, IN_SIZE), np.float32)
        xc[:REAL] = x[c * REAL:(c + 1) * REAL]
        xT_c = np.ascontiguousarray(xc.T.astype(ml_dtypes.bfloat16))
        dv = dinv[c * NCN:(c + 1) * NCN]
        dinv_e_c = np.ascontiguousarray(
            dv.reshape(NTILES, P).T.astype(np.float32))
        dinvs_c = np.ascontiguousarray(
            dv[sig[c]].reshape(NTILES, P).T.astype(np.float32))
        in_maps.append({
            "xT": xT_c,
            "w_enc": np.asarray(W_enc, np.float32).astype(ml_dtypes.bfloat16),
            "b_enc": np.asarray(b_enc, np.float32).reshape(1, -1)
                       .astype(ml_dtypes.bfloat16),
            "w_dec": np.asarray(W_dec, np.float32).astype(ml_dtypes.bfloat16),
            "b_dec": b_dec_eff.reshape(1, -1).astype(ml_dtypes.bfloat16),
            "dinv_e": dinv_e_c,
            "dinvs": dinvs_c,
            "gidx": _wrap_idx(sched["gidx"][c]),
        })
    return sched, in_maps


def kernel(x, W_enc, b_enc, W_dec, b_dec, gcn_bias, edge_index,
           _profile=False):
    key = hash(np.asarray(edge_index).tobytes())
    sched, in_maps = _prepare(x, W_enc, b_enc, W_dec, b_dec, gcn_bias,
                              edge_index)
    if key in _cache:
        nc = _cache[key]
    else:
        nc = _build_nc(sched)
        _cache[key] = nc

    res = run_bass_kernel_spmd(nc, in_maps, core_ids=list(range(NCORES)),
                               trace=_profile)
    sig = sched["sig"]
    outp = np.empty((N, IN_SIZE), np.float32)
    for c in range(NCORES):
        o = np.asarray(res.results[c]["out"], np.float32)
        mask = sig[c] < REAL
        outp[c * REAL + sig[c][mask]] = o[mask]
    if _profile:
        return outp, res
    return outp

